# revision 3
# baseline (speedup 1.0000x reference)
"""Trainium2 Bass kernel for nn_AttnBlock: GroupNorm -> single-head spatial
self-attention (QKV 1x1 convs, softmax over 1024 positions, AV) -> proj 1x1
conv -> residual.

Sharding: data-parallel over batch. B=16 -> 2 batches per NeuronCore x 8
cores; identical NEFF per core; host gathers.

v4: full-fp8 matmul pipeline. Beyond v3 (fp8 score path), the attention
weights at2 = exp(scores) and values vt are stored fp8, so the AV matmul
and the softmax-denominator matmul also run fp8 DoubleRow (2x rate, half
the instructions): AV 64 bf16 matmuls -> 32 DR, denom 16 bf16 -> 8 DR,
~30.7k PE cycles/batch vs v3's ~61k. The error catastrophe v3 measured
for fp8 attention weights (2.2e-2) was CLIPPING, not quantization: with
EXP_BIAS=-1.5 and a x16 evac scale, exp outputs reach e^{5.5}*16 >> 224
and the top softmax weights saturate. Fix: EXP_BIAS=-4 (softmax is
shift-invariant) and no evac scale on at2 -- exp outputs stay <= ~e^3,
and because the denominator matmul consumes the SAME fp8 values as AV,
per-key quantization errors cancel in the ratio. Host numpy model:
v3 config 1.56e-2, v4 config 1.34e-2 absmax-rel (gate 2e-2; HW v3
measured 1.60e-2).

The denominator psum stays open across the whole scores phase and
accumulates per key-tile-pair right after each exp lands, so the
reciprocal fires immediately after the last exp instead of after a
16-matmul tail.

Engine split (Pool/GpSimd cannot touch PSUM, so all psum evacuation is
DVE/ACT): DVE runs GroupNorm bn_stats/bn_aggr + rsqrt chain, reciprocal,
AV-evac normalize (TT mult), proj residual STT; ACT runs exp, t2/vt
evacuations (Identity); Pool runs the SBUF-only h = s*x + b stores
(per-partition tensor_scalar) plus the weight-const DMAs; stores ride
the SP HWDGE queue.
"""

import os
import sys

import numpy as np

for _p in ("/opt/trn_rl_repo", "/root/.axon_site/_ro/trn_rl_repo"):
    if os.path.isdir(_p) and _p not in sys.path:
        sys.path.insert(0, _p)

import concourse.bacc as bacc
import concourse.tile as tile
import concourse.mybir as mybir
from concourse.alu_op_type import AluOpType
from concourse.bass_utils import run_bass_kernel_spmd

B, C, H, W = 16, 512, 32, 32
N = H * W                  # 1024 spatial positions
GROUPS = 32
GS = C // GROUPS           # 16 channels per group
NCORES = 8
BPC = B // NCORES          # batches per core
CT = C // 128              # channel 128-tiles
CP = CT // 2               # channel 256-pair groups (DoubleRow)
NT = N // 128              # position 128-tiles
KP = NT // 2               # key 256-pair groups
NCH = N // 512             # 512-wide query chunks
EPS = 1e-5
ATTN_SCALE = float(C) ** -0.5
SW = 16.0                  # fp8 scale for wv/wp
SM = 32.0                  # fp8 scale for M = Wq^T Wk
EXP_BIAS = -4.0            # keeps exp() outputs <= ~e^3 << fp8 max 224

F32 = mybir.dt.float32
BF16 = mybir.dt.bfloat16
FP8 = mybir.dt.float8e4
I32 = mybir.dt.int32
DR = mybir.MatmulPerfMode.DoubleRow
Act = mybir.ActivationFunctionType

LAST_RESULTS = None        # BassKernelResults of the most recent run (for test.py)

_PROGRAM_CACHE = {}


def _build_program(flags=(), loop_reps=None, bench_internal=False):
    """Build the per-core Bass program.

    loop_reps: if set, wrap the whole per-core body in a hardware For_i loop
    executing it that many times (benchmarking only -- output is identical
    every iteration since the program re-reads xs).

    bench_internal: benchmarking only -- declare all big tensors as Internal
    DRAM (zero-filled by a one-time prologue) so timed calls move no host
    data; the program computes on zeros but executes identical instructions."""
    nc = bacc.Bacc(
        "TRN2",
        target_bir_lowering=False,
        debug=False,
        enable_asserts=False,
        num_devices=NCORES,
    )

    kind = "Internal" if bench_internal else "ExternalInput"

    def din(name, shape, dt=F32):
        return nc.dram_tensor(name, shape, dt, kind=kind).ap()

    xs = din("xs", [BPC, CT, 128, N])
    wm = din("wm2", [CP, 128, 2 * C], FP8)
    wv = din("wv2", [CP, 128, 2 * C], FP8)
    wp = din("wp2", [CP, 128, 2 * C], FP8)
    gnw = din("gnw", [128, CT])
    gnb = din("gnb", [128, CT])
    ones_d = din("ones2", [128, 256], FP8)
    gmat_d = din("gmat", [128, 128])

    out_kind = "Internal" if bench_internal else "ExternalOutput"
    out_d = nc.dram_tensor("out", [BPC, CT, 128, N], F32, kind=out_kind).ap()
    sink_d = (nc.dram_tensor("sink", [1, 4], F32, kind="ExternalOutput").ap()
              if bench_internal else None)

    with tile.TileContext(nc) as tc:
        if bench_internal:
            with tc.tile_pool(name="zfill", bufs=1) as zp:
                zt = zp.tile([128, N], F32, tag="z", name="zt")
                nc.vector.memset(zt, 0.01)
                for b_ in range(BPC):
                    for t_ in range(CT):
                        nc.sync.dma_start(out=xs[b_, t_], in_=zt)
                for w_ in (wm, wv, wp):
                    for cp_ in range(CP):
                        nc.sync.dma_start(out=w_[cp_],
                                          in_=zt.bitcast(FP8)[:, 0:2 * C])
                nc.sync.dma_start(out=gnw, in_=zt[:, 0:CT])
                nc.sync.dma_start(out=gnb, in_=zt[:, 0:CT])
                nc.sync.dma_start(out=ones_d, in_=zt.bitcast(FP8)[:, 0:256])
                nc.sync.dma_start(out=gmat_d, in_=zt[:, 0:128])
                nc.sync.dma_start(out=sink_d, in_=zt[0:1, 0:4])
        _emit(tc, xs, wm, wv, wp, gnw, gnb, ones_d, gmat_d, out_d,
              loop_reps=loop_reps)
    nc.compile()
    return nc


def _r2(ap):
    """[128, 2*F] view as [128, 2, F] for DoubleRow operands."""
    return ap.rearrange("p (two f) -> p two f", two=2)


def _emit(tc, xs, wm, wv, wp, gnw, gnb, ones_d, gmat_d, out_d,
          loop_reps=None):
    nc = tc.nc
    from contextlib import ExitStack
    ctx = ExitStack()
    with ctx:
        consts = ctx.enter_context(tc.tile_pool(name="consts", bufs=1))
        xin = ctx.enter_context(tc.tile_pool(name="xin", bufs=8))
        small = ctx.enter_context(tc.tile_pool(name="small", bufs=4))
        hpool = ctx.enter_context(tc.tile_pool(name="hpool", bufs=4))
        tpool = ctx.enter_context(tc.tile_pool(name="tpool", bufs=4))
        vpool = ctx.enter_context(tc.tile_pool(name="vpool", bufs=8))
        apool = ctx.enter_context(tc.tile_pool(name="apool", bufs=8))
        rpool = ctx.enter_context(tc.tile_pool(name="rpool", bufs=2))
        bnpool = ctx.enter_context(tc.tile_pool(name="bnpool", bufs=1))
        h2pool = ctx.enter_context(tc.tile_pool(name="h2pool", bufs=4))
        psbig = ctx.enter_context(tc.tile_pool(name="psbig", bufs=3, space="PSUM"))
        psdn = ctx.enter_context(tc.tile_pool(name="psdn", bufs=1, space="PSUM"))

        A, V, P = nc.scalar, nc.vector, nc.gpsimd

        def copy_psum(eng, out, in_):
            """psum -> sbuf evacuation with dtype convert on ACT or DVE."""
            if eng is A:
                nc.scalar.activation(out, in_, Act.Identity)
            else:
                eng.tensor_copy(out=out, in_=in_)

        # ---- constants ----
        # gmat/wm ride the Pool SWDGE queue (idle at start, cheap seq);
        # the rest share the SP queue behind the first x tile.
        def load_const(tag, src, shape, dt=F32, eng=None):
            t = consts.tile(shape, dt, tag=tag, name=tag)
            (eng or nc.gpsimd).dma_start(out=t, in_=src)
            return t

        def xload0():
            """batch-0 x spread over the SP/ACT/Pool DMA queues; tile 0
            lands as two half-DMAs so bn_stats chunk 0 starts sooner."""
            xt = [xin.tile([128, N], F32, tag="xt", name="x0t")
                  for _ in range(CT)]
            nc.sync.dma_start(out=xt[0][:, 0:512], in_=xs[0, 0][:, 0:512])
            nc.sync.dma_start(out=xt[0][:, 512:1024], in_=xs[0, 0][:, 512:1024])
            nc.scalar.dma_start(out=xt[1], in_=xs[0, 1])
            nc.gpsimd.dma_start(out=xt[2], in_=xs[0, 2])
            nc.gpsimd.dma_start(out=xt[3], in_=xs[0, 3])
            return xt

        x0_tiles = xload0()
        gmat_sb = load_const("gmat", gmat_d, [128, 128])   # feeds gn matmul
        wm_sb = [load_const(f"wm{cp}", wm[cp], [128, 2 * C], FP8)
                 for cp in range(CP)]
        ebias_sb = consts.tile([128, 1], F32, tag="ebias", name="ebias")
        nc.vector.memset(ebias_sb, EXP_BIAS)

        def late_consts():
            """weights not needed until mid-phase: SP queue after x0t0.
            Doubles as ballast so batch 1's x lands only after batch 0's
            GroupNorm chain is done (keeps the list scheduler from slotting
            bn1 ops between the serial chain0 ops)."""
            wv_sb = [load_const(f"wv{cp}", wv[cp], [128, 2 * C], FP8,
                                eng=nc.sync) for cp in range(CP)]
            gnw_sb = load_const("gnw", gnw, [128, CT], eng=nc.sync)
            gnb_sb = load_const("gnb", gnb, [128, CT], eng=nc.sync)
            wp_sb = [load_const(f"wp{cp}", wp[cp], [128, 2 * C], FP8,
                                eng=nc.sync) for cp in range(CP)]
            ones_sb = load_const("ones", ones_d, [128, 256], FP8, eng=nc.sync)
            return gnw_sb, gnb_sb, wv_sb, wp_sb, ones_sb

        def gn_stats(b, xt, gnw_sb, gnb_sb, gps):
            """GroupNorm stats for batch b on DVE (bn_stats/bn_aggr), then
            the gmat matmul + Newton-rsqrt chain. Returns (sc, bc) ptrs."""
            # bn_stats: [count, mean, count*var] per 512-col chunk (x2 for
            # the even/odd element split); bn_aggr combines into [mean, var]
            bno = bnpool.tile([128, CT, 2, 6], F32, tag="bno", name="bno")
            stat = small.tile([128, 24], F32, tag="stat")
            for t in range(CT):
                for ch in range(2):
                    nc.vector.bn_stats(bno[:, t, ch],
                                       xt[t][:, 512 * ch:512 * (ch + 1)])
            for t in range(CT):
                # mean -> stat[:, t], var -> stat[:, 4+t] (stride-4 out AP)
                nc.vector.bn_aggr(stat[:, t:t + 5:4], bno[:, t])
            mean, var = stat[:, 0:4], stat[:, 4:8]
            m2 = stat[:, 8:12]
            nc.vector.tensor_tensor(m2, mean, mean, AluOpType.mult)
            nc.vector.tensor_tensor(var, var, m2, AluOpType.add)  # E[x^2]
            nc.tensor.matmul(gps, lhsT=gmat_sb, rhs=stat[:, 0:8],
                             start=True, stop=True)
            g = small.tile([128, 20], F32, tag="gst")
            nc.vector.tensor_copy(out=g[:, 0:8], in_=gps)
            gmean, gex2 = g[:, 0:4], g[:, 4:8]
            gm2, ve, y, c = g[:, 8:12], g[:, 12:16], g[:, 16:20], stat[:, 8:12]
            th, sc, bc = stat[:, 12:16], stat[:, 16:20], stat[:, 20:24]
            nc.vector.tensor_tensor(gm2, gmean, gmean, AluOpType.mult)
            nc.vector.scalar_tensor_tensor(out=ve, in0=gex2, scalar=EPS,
                                           in1=gm2, op0=AluOpType.add,
                                           op1=AluOpType.subtract)  # var+eps
            # Newton rsqrt (bit-trick seed + 1 iteration, ~0.2% worst case --
            # far below the fp8 noise floor) on DVE: keeps ACT free of
            # Sqrt/Ln so one table set serves the whole program
            nc.vector.tensor_scalar_mul(th, ve, 0.5)
            nc.vector.tensor_scalar(y.bitcast(I32), ve.bitcast(I32), 1, None,
                                    op0=AluOpType.logical_shift_right)
            nc.vector.tensor_scalar(y.bitcast(I32), y.bitcast(I32),
                                    -1, 0x5f3759df,
                                    op0=AluOpType.mult, op1=AluOpType.add)
            nc.vector.tensor_tensor(c, y, y, AluOpType.mult)
            nc.vector.tensor_tensor(c, th, c, AluOpType.mult)
            nc.vector.tensor_scalar(c, c, -1.0, 1.5,
                                    op0=AluOpType.mult, op1=AluOpType.add)
            nc.vector.tensor_tensor(y, y, c, AluOpType.mult)
            nc.vector.tensor_tensor(sc, y, gnw_sb, AluOpType.mult)      # s
            nc.vector.tensor_tensor(bc, gmean, sc, AluOpType.mult)      # mean*s
            nc.vector.tensor_tensor(bc, gnb_sb, bc, AluOpType.subtract)  # b'
            return sc, bc

        def h_tile(h2t, xt, sc, bc, t, eng):
            """h = s*x + b' for one channel tile -> fp8 DoubleRow layout."""
            dst = h2t[t // 2][:, (t % 2) * N:(t % 2 + 1) * N]
            if eng is A:
                nc.scalar.activation(dst, xt[t], Act.Identity,
                                     scale=sc[:, t:t + 1], bias=bc[:, t:t + 1])
            else:
                eng.tensor_scalar(dst, xt[t], sc[:, t:t + 1], bc[:, t:t + 1],
                                  op0=AluOpType.mult, op1=AluOpType.add)

        def new_h2t():
            return [hpool.tile([128, 2 * N], FP8, tag="h", name="h2t")
                    for _ in range(CP)]

        def t_group(h2t, t2, dt, eng):
            """t = M h for one output channel tile (psum halves nch0|nch1)."""
            dsl = slice(128 * dt, 128 * (dt + 1))
            ps = psbig.tile([128, 1024], F32, tag="ps")
            for cp in range(CP):
                lw = _r2(wm_sb[cp])[:, :, dsl]
                for nch in range(NCH):
                    nsl = slice(512 * nch, 512 * (nch + 1))
                    nc.tensor.matmul(ps[:, 512 * nch:512 * (nch + 1)],
                                     lhsT=lw,
                                     rhs=_r2(h2t[cp])[:, :, nsl],
                                     start=(cp == 0), stop=(cp == CP - 1),
                                     perf_mode=DR)
            copy_psum(eng, t2[dt // 2][:, (dt % 2) * N:(dt % 2 + 1) * N], ps)

        def v_group(h2t, wv_sb, np_, eng):
            """One v^T tile pair (key tiles 2np_, 2np_+1), fp8 out."""
            ps = psbig.tile([128, 1024], F32, tag="ps")
            for i2 in range(2):
                psl = slice(128 * (2 * np_ + i2), 128 * (2 * np_ + i2 + 1))
                for cp in range(CP):
                    nc.tensor.matmul(ps[:, 512 * i2:512 * (i2 + 1)],
                                     lhsT=_r2(h2t[cp])[:, :, psl],
                                     rhs=_r2(wv_sb[cp]),
                                     start=(cp == 0), stop=(cp == CP - 1),
                                     perf_mode=DR)
            vt = vpool.tile([128, 1024], FP8, tag="vt")
            copy_psum(eng, vt, ps)
            return vt

        def scores_pt(h2t, t2, at2, pt, exp_eng=None):
            """scores^T[key, query] = t^T h + exp -> fp8 for one key tile."""
            ksl = slice(128 * pt, 128 * (pt + 1))
            ps = psbig.tile([128, 1024], F32, tag="ps")
            for cp in range(CP):
                lt = _r2(t2[cp])[:, :, ksl]
                for nch in range(NCH):
                    nsl = slice(512 * nch, 512 * (nch + 1))
                    nc.tensor.matmul(ps[:, 512 * nch:512 * (nch + 1)],
                                     lhsT=lt, rhs=_r2(h2t[cp])[:, :, nsl],
                                     start=(cp == 0), stop=(cp == CP - 1),
                                     perf_mode=DR)
            nc.scalar.activation(
                at2[pt // 2][:, (pt % 2) * N:(pt % 2 + 1) * N], ps,
                Act.Exp, scale=ATTN_SCALE / SM, bias=ebias_sb)

        def denom_acc(psd, at2, kp, ones_sb):
            """accumulate one key-tile pair into the open denominator psum
            (fp8 DR all-ones matmul over the SAME quantized at2 AV reads)."""
            lo = _r2(ones_sb)
            for nch in range(NCH):
                nsl = slice(512 * nch, 512 * (nch + 1))
                nc.tensor.matmul(psd[:, 512 * nch:512 * (nch + 1)],
                                 lhsT=lo, rhs=_r2(at2[kp])[:, :, nsl],
                                 start=(kp == 0), stop=(kp == KP - 1),
                                 perf_mode=DR)

        def av_group(at2, vt2, rc, h2q, ct, eng, halves=False):
            """AV (fp8 DR) for one output channel tile; normalize by 1/denom
            on evacuation. halves=True: evacuate 512-col halves separately so
            the tail-batch proj can start on half 0 early."""
            ps = psbig.tile([128, 1024], F32, tag="ps")
            csl = slice(128 * ct, 128 * (ct + 1))
            for kp in range(KP):
                lv = _r2(vt2[kp])[:, :, csl]
                for nch in range(NCH):
                    nsl = slice(512 * nch, 512 * (nch + 1))
                    nc.tensor.matmul(ps[:, 512 * nch:512 * (nch + 1)],
                                     lhsT=lv, rhs=_r2(at2[kp])[:, :, nsl],
                                     start=(kp == 0), stop=(kp == KP - 1),
                                     perf_mode=DR)
            dst = h2q[ct // 2][:, (ct % 2) * N:(ct % 2 + 1) * N]
            if halves:
                for nch in range(NCH):
                    qsl = slice(512 * nch, 512 * (nch + 1))
                    eng.tensor_tensor(dst[:, qsl], ps[:, qsl], rc[:, qsl],
                                      AluOpType.mult)
            else:
                eng.tensor_tensor(dst, ps, rc, AluOpType.mult)

        def proj_group(b, xt, wp_sb, h2q, dt, dma_eng, act_pool=False,
                       pool=None, half_stores=False):
            """proj (fp8 DR) + residual (in place into xt) + store for one
            output channel tile; 512-wide evacuation halves, nch-major so
            half 0 closes after two matmuls.

            act_pool=True (tail batch): evacuate psum via ACT Identity into a
            scratch tile and do the residual add on Pool -- DVE is saturated
            with h2q evacs at the tail, and ACT/Pool are idle post-exp.
            half_stores=True: DMA each 512-col half as soon as its residual
            add lands (shorter store tail)."""
            dsl = slice(128 * dt, 128 * (dt + 1))
            ps = (pool or psbig).tile([128, 1024], F32,
                                      tag="psd" if pool else "ps",
                                      name="pps")
            for nch in range(NCH):
                qsl = slice(512 * nch, 512 * (nch + 1))
                for cp in range(CP):
                    lw = _r2(wp_sb[cp])[:, :, dsl]
                    nc.tensor.matmul(ps[:, 512 * nch:512 * (nch + 1)],
                                     lhsT=lw, rhs=_r2(h2q[cp])[:, :, qsl],
                                     start=(cp == 0), stop=(cp == CP - 1),
                                     perf_mode=DR)
            tmp = None
            if act_pool:
                tmp = vpool.tile([128, 1024], F32, tag="ptmp", name="ptmp")
            for nch in range(NCH):
                qsl = slice(512 * nch, 512 * (nch + 1))
                if act_pool:
                    nc.scalar.activation(tmp[:, qsl], ps[:, qsl], Act.Identity,
                                         scale=1.0 / (SW * SW))
                    nc.gpsimd.tensor_tensor(xt[dt][:, qsl], tmp[:, qsl],
                                            xt[dt][:, qsl], AluOpType.add)
                else:
                    nc.vector.scalar_tensor_tensor(
                        out=xt[dt][:, qsl], in0=ps[:, qsl],
                        scalar=1.0 / (SW * SW), in1=xt[dt][:, qsl],
                        op0=AluOpType.mult, op1=AluOpType.add)
                if half_stores:
                    dma_eng.dma_start(out=out_d[b, dt][:, qsl],
                                      in_=xt[dt][:, qsl])
            if not half_stores:
                dma_eng.dma_start(out=out_d[b, dt], in_=xt[dt])

        def new_at2():
            return [apool.tile([128, 2 * N], FP8, tag="at", name="at2")
                    for _ in range(KP)]

        def new_h2q():
            return [h2pool.tile([128, 2 * N], FP8, tag="h2", name="h2q")
                    for _ in range(CP)]

        def body():
            # Cross-batch software pipeline. Key ordering constraints:
            # - exp (ACT) gates denom->recip->AV, so nothing else sits
            #   between a batch's exps in the ACT stream; vt0 evacs go to
            #   DVE (idle there), vt1 evacs to ACT (post-exp1, DVE is busy
            #   with h2q0/proj0 then).
            # - x1 loads are queued at the END of the SP chain so batch 1's
            #   bn_stats can't get hoisted between batch 0's serial rsqrt
            #   chain ops by the list scheduler.
            x0 = x0_tiles
            gnw_sb, gnb_sb, wv_sb, wp_sb, ones_sb = late_consts()
            # both GroupNorm psums allocated up front: allocating gps1 mid-
            # rotation would chain its WAR to a V0 psum whose evacuation
            # lands ~8us later, strangling the whole batch-1 front
            gps0 = psbig.tile([128, 8], F32, tag="ps", name="gps0")
            gps1 = psbig.tile([128, 8], F32, tag="ps", name="gps1")
            x1 = [xin.tile([128, N], F32, tag="xt", name="x1t")
                  for _ in range(CT)]
            for t in range(CT):
                nc.sync.dma_start(out=x1[t], in_=xs[1, t])
            sc0, bc0 = gn_stats(0, x0, gnw_sb, gnb_sb, gps0)
            # WAR blocker: bn1's stats buffer (bnpool bufs=1) can only be
            # reused after this write, which depends on the end of batch 0's
            # serial rsqrt chain -- keeps the list scheduler from slotting
            # 594ns bn1 ops between the 65ns chain0 ops.
            blocker = bnpool.tile([128, CT, 2, 6], F32, tag="bno",
                                  name="blocker")
            nc.vector.tensor_copy(out=blocker[:, 0, 0, 0:1], in_=bc0[:, 0:1])
            h0 = new_h2t()
            for t, eng in zip(range(CT), (A, V, P, P)):
                h_tile(h0, x0, sc0, bc0, t, eng)
            t20 = [tpool.tile([128, 2 * N], FP8, tag="t", name="t2")
                   for _ in range(CP)]
            for dt in range(CT):
                t_group(h0, t20, dt, A)
            # batch-1 GroupNorm emitted BEFORE the scores span: its tiny
            # gmat matmul must sit early in the PE FIFO -- behind the
            # denominator matmuls (which wait on exp0-pt7) it would jam in
            # the 4-deep wait queue and stall the whole batch-1 front.
            sc1, bc1 = gn_stats(1, x1, gnw_sb, gnb_sb, gps1)
            h1 = new_h2t()
            for t in range(CT):
                h_tile(h1, x1, sc1, bc1, t, P)
            at0, vt0 = new_at2(), []
            psd0 = psdn.tile([128, 1024], F32, tag="psd")
            for pt in range(NT):
                scores_pt(h0, t20, at0, pt)
                if pt % 2 == 1:
                    kp = pt // 2
                    denom_acc(psd0, at0, kp, ones_sb)
                    vt0.append(v_group(h0, wv_sb, kp, V))
            rc0 = rpool.tile([128, 1024], F32, tag="rc")
            nc.vector.reciprocal(out=rc0, in_=psd0)
            t21 = [tpool.tile([128, 2 * N], FP8, tag="t", name="t2")
                   for _ in range(CP)]
            for dt in range(CT):
                t_group(h1, t21, dt, A)                  # ACT: right after exp0
            h2q0 = new_h2q()
            for ct in range(CT):
                av_group(at0, vt0, rc0, h2q0, ct, V)
            at1, vt1 = new_at2(), []
            psd1 = psdn.tile([128, 1024], F32, tag="psd")
            for pt in range(NT):
                scores_pt(h1, t21, at1, pt)
                if pt % 2 == 1:
                    kp = pt // 2
                    denom_acc(psd1, at1, kp, ones_sb)
                    vt1.append(v_group(h1, wv_sb, kp, V))
            # proj0 evacuates via ACT+Pool (emitted after scores1 so its ACT
            # evacs queue behind exp1, not in front of it); stores ride SP
            for dt in range(CT):
                proj_group(0, x0, wp_sb, h2q0, dt, dma_eng=nc.sync,
                           act_pool=True, pool=psdn)
            rc1 = rpool.tile([128, 1024], F32, tag="rc")
            nc.vector.reciprocal(out=rc1, in_=psd1)
            h2q1 = new_h2q()
            for ct in range(CT):
                av_group(at1, vt1, rc1, h2q1, ct, V, halves=True)
            # tail: dt0/dt2 evacuate via ACT+Pool, dt1/dt3 via DVE STT --
            # both evacuation chains drain the last batch in parallel
            store1 = {0: nc.scalar, 1: nc.sync, 2: nc.scalar, 3: nc.sync}
            for dt in range(CT):
                proj_group(1, x1, wp_sb, h2q1, dt, dma_eng=store1[dt],
                           act_pool=(dt % 2 == 0), half_stores=True)

        if loop_reps is None:
            body()
        else:
            with tc.For_i(0, loop_reps, 1):
                body()


def _pack_w(w, scale):
    """W[d_out, c_in] -> DoubleRow stationary layout [CP, 128, 2*C] fp8,
    scaled. [cp][p, ko*C + d] = W.T[cp*256 + ko*128 + p, d] * scale."""
    f8 = mybir.dt.np(FP8)
    wT = np.ascontiguousarray(np.asarray(w, np.float32).T) * scale
    wT = np.clip(wT, -224.0, 224.0)
    return np.ascontiguousarray(
        wT.reshape(CP, 2, 128, C).transpose(0, 2, 1, 3).reshape(CP, 128, 2 * C)
    ).astype(f8)


def _prep_inputs(x, gn_w, gn_b, q_w, q_b, k_w, k_b, v_w, v_b, p_w, p_b):
    f = np.float32
    for name, bias in (("q_b", q_b), ("k_b", k_b), ("v_b", v_b), ("p_b", p_b)):
        if np.any(np.asarray(bias)):
            raise NotImplementedError(f"nonzero {name} not supported")
    x = np.ascontiguousarray(np.asarray(x, f)).reshape(B, CT, 128, N)
    m = np.asarray(q_w, f).T @ np.asarray(k_w, f)   # scores = h^T (M h)
    base = {
        "wm2": _pack_w(m, SM),
        "wv2": _pack_w(v_w, SW),
        "wp2": _pack_w(p_w, SW),
        "gnw": np.ascontiguousarray(np.asarray(gn_w, f).reshape(CT, 128).T),
        "gnb": np.ascontiguousarray(np.asarray(gn_b, f).reshape(CT, 128).T),
        "ones2": np.ones((128, 256), f).astype(mybir.dt.np(FP8)),
        # block-diagonal group-averaging matrix: G[p, m] = 1/GS iff
        # p//GS == m//GS (bn stats are already per-channel means over N,
        # so only the 16-channel group average remains)
        "gmat": np.ascontiguousarray(
            np.kron(np.eye(128 // GS, dtype=f), np.ones((GS, GS), f)) / GS),
    }
    return x, base, ()


def kernel(x, temb, gn_w, gn_b, q_w, q_b, k_w, k_b, v_w, v_b, p_w, p_b):
    global LAST_RESULTS
    del temb  # unused by the reference module
    x_r, base, flags = _prep_inputs(x, gn_w, gn_b, q_w, q_b, k_w, k_b,
                                    v_w, v_b, p_w, p_b)
    if flags not in _PROGRAM_CACHE:
        _PROGRAM_CACHE[flags] = _build_program(flags)
    nc = _PROGRAM_CACHE[flags]

    in_maps = [dict(base, xs=np.ascontiguousarray(x_r[BPC * i: BPC * (i + 1)]))
               for i in range(NCORES)]
    res = run_bass_kernel_spmd(nc, in_maps, core_ids=list(range(NCORES)))
    LAST_RESULTS = res
    out = np.concatenate([r["out"] for r in res.results], axis=0)
    return np.ascontiguousarray(out.reshape(B, C, H, W).astype(np.float32))


# revision 4
# speedup vs baseline: 1.0608x; 1.0608x over previous
"""Trainium2 Bass kernel for nn_AttnBlock: GroupNorm -> single-head spatial
self-attention (QKV 1x1 convs, softmax over 1024 positions, AV) -> proj 1x1
conv -> residual.

Sharding: data-parallel over batch. B=16 -> 2 batches per NeuronCore x 8
cores; identical NEFF per core; host gathers.

v4: full-fp8 matmul pipeline. Beyond v3 (fp8 score path), the attention
weights at2 = exp(scores) and values vt are stored fp8, so the AV matmul
and the softmax-denominator matmul also run fp8 DoubleRow (2x rate, half
the instructions): AV 64 bf16 matmuls -> 32 DR, denom 16 bf16 -> 8 DR,
~30.7k PE cycles/batch vs v3's ~61k. The error catastrophe v3 measured
for fp8 attention weights (2.2e-2) was CLIPPING, not quantization: with
EXP_BIAS=-1.5 and a x16 evac scale, exp outputs reach e^{5.5}*16 >> 224
and the top softmax weights saturate. Fix: EXP_BIAS=-4 (softmax is
shift-invariant) and no evac scale on at2 -- exp outputs stay <= ~e^3,
and because the denominator matmul consumes the SAME fp8 values as AV,
per-key quantization errors cancel in the ratio. Host numpy model:
v3 config 1.56e-2, v4 config 1.34e-2 absmax-rel (gate 2e-2; HW v3
measured 1.60e-2).

The denominator psum stays open across the whole scores phase and
accumulates per key-tile-pair right after each exp lands, so the
reciprocal fires immediately after the last exp instead of after a
16-matmul tail.

Engine split (Pool/GpSimd cannot touch PSUM, so all psum evacuation is
DVE/ACT): DVE runs GroupNorm bn_stats/bn_aggr + rsqrt chain, reciprocal,
vt0 evacuations, AV-evac normalize (TT mult), and the dt1/dt3 tail STTs;
ACT runs exp, t2 evacuations, vt1 evacuations, and the proj0 + dt0/dt2
proj1 evacuations (Identity x 1/SW^2, residual added on Pool); Pool runs
the SBUF-only h = s*x + b stores plus weight-const DMAs. Batch-1 GroupNorm
is emitted before the scores span (its gmat matmul otherwise jams behind
exp-blocked denominator matmuls in the PE wait queue), a pool-WAR blocker
tile keeps bn1 from interleaving batch 0's serial rsqrt chain, both gn
psums are pre-allocated (a mid-rotation alloc chains to a V-psum evac
~8us out), proj0 psums live in the denominator pool to keep AV1's psbig
slots free, and the tail stores 512-col halves over the SP+ACT queues.
"""

import os
import sys

import numpy as np

for _p in ("/opt/trn_rl_repo", "/root/.axon_site/_ro/trn_rl_repo"):
    if os.path.isdir(_p) and _p not in sys.path:
        sys.path.insert(0, _p)

import concourse.bacc as bacc
import concourse.tile as tile
import concourse.mybir as mybir
from concourse.alu_op_type import AluOpType
from concourse.bass_utils import run_bass_kernel_spmd

B, C, H, W = 16, 512, 32, 32
N = H * W                  # 1024 spatial positions
GROUPS = 32
GS = C // GROUPS           # 16 channels per group
NCORES = 8
BPC = B // NCORES          # batches per core
CT = C // 128              # channel 128-tiles
CP = CT // 2               # channel 256-pair groups (DoubleRow)
NT = N // 128              # position 128-tiles
KP = NT // 2               # key 256-pair groups
NCH = N // 512             # 512-wide query chunks
EPS = 1e-5
ATTN_SCALE = float(C) ** -0.5
SW = 16.0                  # fp8 scale for wv/wp
SM = 32.0                  # fp8 scale for M = Wq^T Wk
EXP_BIAS = -4.0            # keeps exp() outputs <= ~e^3 << fp8 max 224

F32 = mybir.dt.float32
BF16 = mybir.dt.bfloat16
FP8 = mybir.dt.float8e4
I32 = mybir.dt.int32
DR = mybir.MatmulPerfMode.DoubleRow
Act = mybir.ActivationFunctionType

LAST_RESULTS = None        # BassKernelResults of the most recent run (for test.py)

_PROGRAM_CACHE = {}


def _build_program(flags=(), loop_reps=None, bench_internal=False):
    """Build the per-core Bass program.

    loop_reps: if set, wrap the whole per-core body in a hardware For_i loop
    executing it that many times (benchmarking only -- output is identical
    every iteration since the program re-reads xs).

    bench_internal: benchmarking only -- declare all big tensors as Internal
    DRAM (zero-filled by a one-time prologue) so timed calls move no host
    data; the program computes on zeros but executes identical instructions."""
    nc = bacc.Bacc(
        "TRN2",
        target_bir_lowering=False,
        debug=False,
        enable_asserts=False,
        num_devices=NCORES,
    )

    kind = "Internal" if bench_internal else "ExternalInput"

    def din(name, shape, dt=F32):
        return nc.dram_tensor(name, shape, dt, kind=kind).ap()

    xs = din("xs", [BPC, CT, 128, N])
    wm = din("wm2", [CP, 128, 2 * C], FP8)
    wv = din("wv2", [CP, 128, 2 * C], FP8)
    wp = din("wp2", [CP, 128, 2 * C], FP8)
    gnw = din("gnw", [128, CT])
    gnb = din("gnb", [128, CT])
    ones_d = din("ones2", [128, 256], FP8)
    gmat_d = din("gmat", [128, 128])

    out_kind = "Internal" if bench_internal else "ExternalOutput"
    out_d = nc.dram_tensor("out", [BPC, CT, 128, N], F32, kind=out_kind).ap()
    sink_d = (nc.dram_tensor("sink", [1, 4], F32, kind="ExternalOutput").ap()
              if bench_internal else None)

    with tile.TileContext(nc) as tc:
        if bench_internal:
            with tc.tile_pool(name="zfill", bufs=1) as zp:
                zt = zp.tile([128, N], F32, tag="z", name="zt")
                nc.vector.memset(zt, 0.01)
                for b_ in range(BPC):
                    for t_ in range(CT):
                        nc.sync.dma_start(out=xs[b_, t_], in_=zt)
                for w_ in (wm, wv, wp):
                    for cp_ in range(CP):
                        nc.sync.dma_start(out=w_[cp_],
                                          in_=zt.bitcast(FP8)[:, 0:2 * C])
                nc.sync.dma_start(out=gnw, in_=zt[:, 0:CT])
                nc.sync.dma_start(out=gnb, in_=zt[:, 0:CT])
                nc.sync.dma_start(out=ones_d, in_=zt.bitcast(FP8)[:, 0:256])
                nc.sync.dma_start(out=gmat_d, in_=zt[:, 0:128])
                nc.sync.dma_start(out=sink_d, in_=zt[0:1, 0:4])
        _emit(tc, xs, wm, wv, wp, gnw, gnb, ones_d, gmat_d, out_d,
              loop_reps=loop_reps)
    nc.compile()
    return nc


def _r2(ap):
    """[128, 2*F] view as [128, 2, F] for DoubleRow operands."""
    return ap.rearrange("p (two f) -> p two f", two=2)


def _emit(tc, xs, wm, wv, wp, gnw, gnb, ones_d, gmat_d, out_d,
          loop_reps=None):
    nc = tc.nc
    from contextlib import ExitStack
    ctx = ExitStack()
    with ctx:
        consts = ctx.enter_context(tc.tile_pool(name="consts", bufs=1))
        xin = ctx.enter_context(tc.tile_pool(name="xin", bufs=8))
        small = ctx.enter_context(tc.tile_pool(name="small", bufs=4))
        hpool = ctx.enter_context(tc.tile_pool(name="hpool", bufs=4))
        tpool = ctx.enter_context(tc.tile_pool(name="tpool", bufs=4))
        vpool = ctx.enter_context(tc.tile_pool(name="vpool", bufs=8))
        apool = ctx.enter_context(tc.tile_pool(name="apool", bufs=8))
        rpool = ctx.enter_context(tc.tile_pool(name="rpool", bufs=2))
        bnpool = ctx.enter_context(tc.tile_pool(name="bnpool", bufs=1))
        h2pool = ctx.enter_context(tc.tile_pool(name="h2pool", bufs=4))
        psbig = ctx.enter_context(tc.tile_pool(name="psbig", bufs=3, space="PSUM"))
        psdn = ctx.enter_context(tc.tile_pool(name="psdn", bufs=1, space="PSUM"))

        A, V, P = nc.scalar, nc.vector, nc.gpsimd

        def copy_psum(eng, out, in_):
            """psum -> sbuf evacuation with dtype convert on ACT or DVE."""
            if eng is A:
                nc.scalar.activation(out, in_, Act.Identity)
            else:
                eng.tensor_copy(out=out, in_=in_)

        # ---- constants ----
        # gmat/wm ride the Pool SWDGE queue (idle at start, cheap seq);
        # the rest share the SP queue behind the first x tile.
        def load_const(tag, src, shape, dt=F32, eng=None):
            t = consts.tile(shape, dt, tag=tag, name=tag)
            (eng or nc.gpsimd).dma_start(out=t, in_=src)
            return t

        def xload0():
            """batch-0 x spread over the SP/ACT/Pool DMA queues; tile 0
            lands as two half-DMAs so bn_stats chunk 0 starts sooner."""
            xt = [xin.tile([128, N], F32, tag="xt", name="x0t")
                  for _ in range(CT)]
            nc.sync.dma_start(out=xt[0][:, 0:512], in_=xs[0, 0][:, 0:512])
            nc.sync.dma_start(out=xt[0][:, 512:1024], in_=xs[0, 0][:, 512:1024])
            nc.scalar.dma_start(out=xt[1], in_=xs[0, 1])
            nc.gpsimd.dma_start(out=xt[2], in_=xs[0, 2])
            nc.gpsimd.dma_start(out=xt[3], in_=xs[0, 3])
            return xt

        x0_tiles = xload0()
        gmat_sb = load_const("gmat", gmat_d, [128, 128])   # feeds gn matmul
        wm_sb = [load_const(f"wm{cp}", wm[cp], [128, 2 * C], FP8)
                 for cp in range(CP)]
        ebias_sb = consts.tile([128, 1], F32, tag="ebias", name="ebias")
        nc.vector.memset(ebias_sb, EXP_BIAS)

        def late_consts():
            """weights not needed until mid-phase: SP queue after x0t0.
            Doubles as ballast so batch 1's x lands only after batch 0's
            GroupNorm chain is done (keeps the list scheduler from slotting
            bn1 ops between the serial chain0 ops)."""
            wv_sb = [load_const(f"wv{cp}", wv[cp], [128, 2 * C], FP8,
                                eng=nc.sync) for cp in range(CP)]
            gnw_sb = load_const("gnw", gnw, [128, CT], eng=nc.sync)
            gnb_sb = load_const("gnb", gnb, [128, CT], eng=nc.sync)
            wp_sb = [load_const(f"wp{cp}", wp[cp], [128, 2 * C], FP8,
                                eng=nc.sync) for cp in range(CP)]
            ones_sb = load_const("ones", ones_d, [128, 256], FP8, eng=nc.sync)
            return gnw_sb, gnb_sb, wv_sb, wp_sb, ones_sb

        def gn_stats(b, xt, gnw_sb, gnb_sb, gps):
            """GroupNorm stats for batch b on DVE (bn_stats/bn_aggr), then
            the gmat matmul + Newton-rsqrt chain. Returns (sc, bc) ptrs."""
            # bn_stats: [count, mean, count*var] per 512-col chunk (x2 for
            # the even/odd element split); bn_aggr combines into [mean, var]
            bno = bnpool.tile([128, CT, 2, 6], F32, tag="bno", name="bno")
            stat = small.tile([128, 24], F32, tag="stat")
            for t in range(CT):
                for ch in range(2):
                    nc.vector.bn_stats(bno[:, t, ch],
                                       xt[t][:, 512 * ch:512 * (ch + 1)])
            for t in range(CT):
                # mean -> stat[:, t], var -> stat[:, 4+t] (stride-4 out AP)
                nc.vector.bn_aggr(stat[:, t:t + 5:4], bno[:, t])
            mean, var = stat[:, 0:4], stat[:, 4:8]
            m2 = stat[:, 8:12]
            nc.vector.tensor_tensor(m2, mean, mean, AluOpType.mult)
            nc.vector.tensor_tensor(var, var, m2, AluOpType.add)  # E[x^2]
            nc.tensor.matmul(gps, lhsT=gmat_sb, rhs=stat[:, 0:8],
                             start=True, stop=True)
            g = small.tile([128, 20], F32, tag="gst")
            nc.vector.tensor_copy(out=g[:, 0:8], in_=gps)
            gmean, gex2 = g[:, 0:4], g[:, 4:8]
            gm2, ve, y, c = g[:, 8:12], g[:, 12:16], g[:, 16:20], stat[:, 8:12]
            th, sc, bc = stat[:, 12:16], stat[:, 16:20], stat[:, 20:24]
            nc.vector.tensor_tensor(gm2, gmean, gmean, AluOpType.mult)
            nc.vector.scalar_tensor_tensor(out=ve, in0=gex2, scalar=EPS,
                                           in1=gm2, op0=AluOpType.add,
                                           op1=AluOpType.subtract)  # var+eps
            # Newton rsqrt (bit-trick seed + 1 iteration, ~0.2% worst case --
            # far below the fp8 noise floor) on DVE: keeps ACT free of
            # Sqrt/Ln so one table set serves the whole program
            nc.vector.tensor_scalar_mul(th, ve, 0.5)
            nc.vector.tensor_scalar(y.bitcast(I32), ve.bitcast(I32), 1, None,
                                    op0=AluOpType.logical_shift_right)
            nc.vector.tensor_scalar(y.bitcast(I32), y.bitcast(I32),
                                    -1, 0x5f3759df,
                                    op0=AluOpType.mult, op1=AluOpType.add)
            nc.vector.tensor_tensor(c, y, y, AluOpType.mult)
            nc.vector.tensor_tensor(c, th, c, AluOpType.mult)
            nc.vector.tensor_scalar(c, c, -1.0, 1.5,
                                    op0=AluOpType.mult, op1=AluOpType.add)
            nc.vector.tensor_tensor(y, y, c, AluOpType.mult)
            nc.vector.tensor_tensor(sc, y, gnw_sb, AluOpType.mult)      # s
            nc.vector.tensor_tensor(bc, gmean, sc, AluOpType.mult)      # mean*s
            nc.vector.tensor_tensor(bc, gnb_sb, bc, AluOpType.subtract)  # b'
            return sc, bc

        def h_tile(h2t, xt, sc, bc, t, eng):
            """h = s*x + b' for one channel tile -> fp8 DoubleRow layout."""
            dst = h2t[t // 2][:, (t % 2) * N:(t % 2 + 1) * N]
            if eng is A:
                nc.scalar.activation(dst, xt[t], Act.Identity,
                                     scale=sc[:, t:t + 1], bias=bc[:, t:t + 1])
            else:
                eng.tensor_scalar(dst, xt[t], sc[:, t:t + 1], bc[:, t:t + 1],
                                  op0=AluOpType.mult, op1=AluOpType.add)

        def new_h2t():
            return [hpool.tile([128, 2 * N], FP8, tag="h", name="h2t")
                    for _ in range(CP)]

        def t_group(h2t, t2, dt, eng):
            """t = M h for one output channel tile (psum halves nch0|nch1)."""
            dsl = slice(128 * dt, 128 * (dt + 1))
            ps = psbig.tile([128, 1024], F32, tag="ps")
            for cp in range(CP):
                lw = _r2(wm_sb[cp])[:, :, dsl]
                for nch in range(NCH):
                    nsl = slice(512 * nch, 512 * (nch + 1))
                    nc.tensor.matmul(ps[:, 512 * nch:512 * (nch + 1)],
                                     lhsT=lw,
                                     rhs=_r2(h2t[cp])[:, :, nsl],
                                     start=(cp == 0), stop=(cp == CP - 1),
                                     perf_mode=DR)
            copy_psum(eng, t2[dt // 2][:, (dt % 2) * N:(dt % 2 + 1) * N], ps)

        def v_group(h2t, wv_sb, np_, eng):
            """One v^T tile pair (key tiles 2np_, 2np_+1), fp8 out."""
            ps = psbig.tile([128, 1024], F32, tag="ps")
            for i2 in range(2):
                psl = slice(128 * (2 * np_ + i2), 128 * (2 * np_ + i2 + 1))
                for cp in range(CP):
                    nc.tensor.matmul(ps[:, 512 * i2:512 * (i2 + 1)],
                                     lhsT=_r2(h2t[cp])[:, :, psl],
                                     rhs=_r2(wv_sb[cp]),
                                     start=(cp == 0), stop=(cp == CP - 1),
                                     perf_mode=DR)
            vt = vpool.tile([128, 1024], FP8, tag="vt")
            copy_psum(eng, vt, ps)
            return vt

        def scores_pt(h2t, t2, at2, pt, exp_eng=None):
            """scores^T[key, query] = t^T h + exp -> fp8 for one key tile."""
            ksl = slice(128 * pt, 128 * (pt + 1))
            ps = psbig.tile([128, 1024], F32, tag="ps")
            for cp in range(CP):
                lt = _r2(t2[cp])[:, :, ksl]
                for nch in range(NCH):
                    nsl = slice(512 * nch, 512 * (nch + 1))
                    nc.tensor.matmul(ps[:, 512 * nch:512 * (nch + 1)],
                                     lhsT=lt, rhs=_r2(h2t[cp])[:, :, nsl],
                                     start=(cp == 0), stop=(cp == CP - 1),
                                     perf_mode=DR)
            nc.scalar.activation(
                at2[pt // 2][:, (pt % 2) * N:(pt % 2 + 1) * N], ps,
                Act.Exp, scale=ATTN_SCALE / SM, bias=ebias_sb)

        def denom_acc(psd, at2, kp, ones_sb):
            """accumulate one key-tile pair into the open denominator psum
            (fp8 DR all-ones matmul over the SAME quantized at2 AV reads)."""
            lo = _r2(ones_sb)
            for nch in range(NCH):
                nsl = slice(512 * nch, 512 * (nch + 1))
                nc.tensor.matmul(psd[:, 512 * nch:512 * (nch + 1)],
                                 lhsT=lo, rhs=_r2(at2[kp])[:, :, nsl],
                                 start=(kp == 0), stop=(kp == KP - 1),
                                 perf_mode=DR)

        def av_group(at2, vt2, rc, h2q, ct, eng, halves=False):
            """AV (fp8 DR) for one output channel tile; normalize by 1/denom
            on evacuation. halves=True: evacuate 512-col halves separately so
            the tail-batch proj can start on half 0 early."""
            ps = psbig.tile([128, 1024], F32, tag="ps")
            csl = slice(128 * ct, 128 * (ct + 1))
            for kp in range(KP):
                lv = _r2(vt2[kp])[:, :, csl]
                for nch in range(NCH):
                    nsl = slice(512 * nch, 512 * (nch + 1))
                    nc.tensor.matmul(ps[:, 512 * nch:512 * (nch + 1)],
                                     lhsT=lv, rhs=_r2(at2[kp])[:, :, nsl],
                                     start=(kp == 0), stop=(kp == KP - 1),
                                     perf_mode=DR)
            dst = h2q[ct // 2][:, (ct % 2) * N:(ct % 2 + 1) * N]
            if halves:
                for nch in range(NCH):
                    qsl = slice(512 * nch, 512 * (nch + 1))
                    eng.tensor_tensor(dst[:, qsl], ps[:, qsl], rc[:, qsl],
                                      AluOpType.mult)
            else:
                eng.tensor_tensor(dst, ps, rc, AluOpType.mult)

        def proj_group(b, xt, wp_sb, h2q, dt, dma_eng, act_pool=False,
                       pool=None, half_stores=False):
            """proj (fp8 DR) + residual (in place into xt) + store for one
            output channel tile; 512-wide evacuation halves, nch-major so
            half 0 closes after two matmuls.

            act_pool=True (tail batch): evacuate psum via ACT Identity into a
            scratch tile and do the residual add on Pool -- DVE is saturated
            with h2q evacs at the tail, and ACT/Pool are idle post-exp.
            half_stores=True: DMA each 512-col half as soon as its residual
            add lands (shorter store tail)."""
            dsl = slice(128 * dt, 128 * (dt + 1))
            ps = (pool or psbig).tile([128, 1024], F32,
                                      tag="psd" if pool else "ps",
                                      name="pps")
            for nch in range(NCH):
                qsl = slice(512 * nch, 512 * (nch + 1))
                for cp in range(CP):
                    lw = _r2(wp_sb[cp])[:, :, dsl]
                    nc.tensor.matmul(ps[:, 512 * nch:512 * (nch + 1)],
                                     lhsT=lw, rhs=_r2(h2q[cp])[:, :, qsl],
                                     start=(cp == 0), stop=(cp == CP - 1),
                                     perf_mode=DR)
            tmp = None
            if act_pool:
                tmp = vpool.tile([128, 1024], F32, tag="ptmp", name="ptmp")
            for nch in range(NCH):
                qsl = slice(512 * nch, 512 * (nch + 1))
                if act_pool:
                    nc.scalar.activation(tmp[:, qsl], ps[:, qsl], Act.Identity,
                                         scale=1.0 / (SW * SW))
                    nc.gpsimd.tensor_tensor(xt[dt][:, qsl], tmp[:, qsl],
                                            xt[dt][:, qsl], AluOpType.add)
                else:
                    nc.vector.scalar_tensor_tensor(
                        out=xt[dt][:, qsl], in0=ps[:, qsl],
                        scalar=1.0 / (SW * SW), in1=xt[dt][:, qsl],
                        op0=AluOpType.mult, op1=AluOpType.add)
                if half_stores:
                    dma_eng.dma_start(out=out_d[b, dt][:, qsl],
                                      in_=xt[dt][:, qsl])
            if not half_stores:
                dma_eng.dma_start(out=out_d[b, dt], in_=xt[dt])

        def new_at2():
            return [apool.tile([128, 2 * N], FP8, tag="at", name="at2")
                    for _ in range(KP)]

        def new_h2q():
            return [h2pool.tile([128, 2 * N], FP8, tag="h2", name="h2q")
                    for _ in range(CP)]

        def body():
            # Cross-batch software pipeline. Key ordering constraints:
            # - exp (ACT) gates denom->recip->AV, so nothing else sits
            #   between a batch's exps in the ACT stream; vt0 evacs go to
            #   DVE (idle there), vt1 evacs to ACT (post-exp1, DVE is busy
            #   with h2q0/proj0 then).
            # - x1 loads are queued at the END of the SP chain so batch 1's
            #   bn_stats can't get hoisted between batch 0's serial rsqrt
            #   chain ops by the list scheduler.
            x0 = x0_tiles
            gnw_sb, gnb_sb, wv_sb, wp_sb, ones_sb = late_consts()
            # both GroupNorm psums allocated up front: allocating gps1 mid-
            # rotation would chain its WAR to a V0 psum whose evacuation
            # lands ~8us later, strangling the whole batch-1 front
            gps0 = psbig.tile([128, 8], F32, tag="ps", name="gps0")
            gps1 = psbig.tile([128, 8], F32, tag="ps", name="gps1")
            x1 = [xin.tile([128, N], F32, tag="xt", name="x1t")
                  for _ in range(CT)]
            for t in range(CT):
                nc.sync.dma_start(out=x1[t], in_=xs[1, t])
            sc0, bc0 = gn_stats(0, x0, gnw_sb, gnb_sb, gps0)
            # WAR blocker: bn1's stats buffer (bnpool bufs=1) can only be
            # reused after this write, which depends on the end of batch 0's
            # serial rsqrt chain -- keeps the list scheduler from slotting
            # 594ns bn1 ops between the 65ns chain0 ops.
            blocker = bnpool.tile([128, CT, 2, 6], F32, tag="bno",
                                  name="blocker")
            nc.vector.tensor_copy(out=blocker[:, 0, 0, 0:1], in_=bc0[:, 0:1])
            h0 = new_h2t()
            for t, eng in zip(range(CT), (A, V, P, P)):
                h_tile(h0, x0, sc0, bc0, t, eng)
            t20 = [tpool.tile([128, 2 * N], FP8, tag="t", name="t2")
                   for _ in range(CP)]
            for dt in range(CT):
                t_group(h0, t20, dt, A)
            # batch-1 GroupNorm emitted BEFORE the scores span: its tiny
            # gmat matmul must sit early in the PE FIFO -- behind the
            # denominator matmuls (which wait on exp0-pt7) it would jam in
            # the 4-deep wait queue and stall the whole batch-1 front.
            sc1, bc1 = gn_stats(1, x1, gnw_sb, gnb_sb, gps1)
            h1 = new_h2t()
            for t in range(CT):
                h_tile(h1, x1, sc1, bc1, t, P)
            at0, vt0 = new_at2(), []
            psd0 = psdn.tile([128, 1024], F32, tag="psd")
            for pt in range(NT):
                scores_pt(h0, t20, at0, pt)
                if pt % 2 == 1:
                    kp = pt // 2
                    denom_acc(psd0, at0, kp, ones_sb)
                    vt0.append(v_group(h0, wv_sb, kp, V))
            rc0 = rpool.tile([128, 1024], F32, tag="rc")
            nc.vector.reciprocal(out=rc0, in_=psd0)
            t21 = [tpool.tile([128, 2 * N], FP8, tag="t", name="t2")
                   for _ in range(CP)]
            for dt in range(CT):
                t_group(h1, t21, dt, A)                  # ACT: right after exp0
            h2q0 = new_h2q()
            for ct in range(CT):
                av_group(at0, vt0, rc0, h2q0, ct, V)
            at1, vt1 = new_at2(), []
            psd1 = psdn.tile([128, 1024], F32, tag="psd")
            for pt in range(NT):
                scores_pt(h1, t21, at1, pt)
                if pt % 2 == 1:
                    kp = pt // 2
                    denom_acc(psd1, at1, kp, ones_sb)
                    vt1.append(v_group(h1, wv_sb, kp, V))
            # proj0 evacuates via ACT+Pool (emitted after scores1 so its ACT
            # evacs queue behind exp1, not in front of it); stores ride SP
            for dt in range(CT):
                proj_group(0, x0, wp_sb, h2q0, dt, dma_eng=nc.sync,
                           act_pool=True, pool=psdn)
            rc1 = rpool.tile([128, 1024], F32, tag="rc")
            nc.vector.reciprocal(out=rc1, in_=psd1)
            h2q1 = new_h2q()
            for ct in range(CT):
                av_group(at1, vt1, rc1, h2q1, ct, V, halves=True)
            # tail: dt0/dt2 evacuate via ACT+Pool, dt1/dt3 via DVE STT --
            # both evacuation chains drain the last batch in parallel
            store1 = {0: nc.scalar, 1: nc.sync, 2: nc.scalar, 3: nc.sync}
            for dt in range(CT):
                proj_group(1, x1, wp_sb, h2q1, dt, dma_eng=store1[dt],
                           act_pool=(dt % 2 == 0), half_stores=True)

        if loop_reps is None:
            body()
        else:
            with tc.For_i(0, loop_reps, 1):
                body()


def _pack_w(w, scale):
    """W[d_out, c_in] -> DoubleRow stationary layout [CP, 128, 2*C] fp8,
    scaled. [cp][p, ko*C + d] = W.T[cp*256 + ko*128 + p, d] * scale."""
    f8 = mybir.dt.np(FP8)
    wT = np.ascontiguousarray(np.asarray(w, np.float32).T) * scale
    wT = np.clip(wT, -224.0, 224.0)
    return np.ascontiguousarray(
        wT.reshape(CP, 2, 128, C).transpose(0, 2, 1, 3).reshape(CP, 128, 2 * C)
    ).astype(f8)


def _prep_inputs(x, gn_w, gn_b, q_w, q_b, k_w, k_b, v_w, v_b, p_w, p_b):
    f = np.float32
    for name, bias in (("q_b", q_b), ("k_b", k_b), ("v_b", v_b), ("p_b", p_b)):
        if np.any(np.asarray(bias)):
            raise NotImplementedError(f"nonzero {name} not supported")
    x = np.ascontiguousarray(np.asarray(x, f)).reshape(B, CT, 128, N)
    m = np.asarray(q_w, f).T @ np.asarray(k_w, f)   # scores = h^T (M h)
    base = {
        "wm2": _pack_w(m, SM),
        "wv2": _pack_w(v_w, SW),
        "wp2": _pack_w(p_w, SW),
        "gnw": np.ascontiguousarray(np.asarray(gn_w, f).reshape(CT, 128).T),
        "gnb": np.ascontiguousarray(np.asarray(gn_b, f).reshape(CT, 128).T),
        "ones2": np.ones((128, 256), f).astype(mybir.dt.np(FP8)),
        # block-diagonal group-averaging matrix: G[p, m] = 1/GS iff
        # p//GS == m//GS (bn stats are already per-channel means over N,
        # so only the 16-channel group average remains)
        "gmat": np.ascontiguousarray(
            np.kron(np.eye(128 // GS, dtype=f), np.ones((GS, GS), f)) / GS),
    }
    return x, base, ()


def kernel(x, temb, gn_w, gn_b, q_w, q_b, k_w, k_b, v_w, v_b, p_w, p_b):
    global LAST_RESULTS
    del temb  # unused by the reference module
    x_r, base, flags = _prep_inputs(x, gn_w, gn_b, q_w, q_b, k_w, k_b,
                                    v_w, v_b, p_w, p_b)
    if flags not in _PROGRAM_CACHE:
        _PROGRAM_CACHE[flags] = _build_program(flags)
    nc = _PROGRAM_CACHE[flags]

    in_maps = [dict(base, xs=np.ascontiguousarray(x_r[BPC * i: BPC * (i + 1)]))
               for i in range(NCORES)]
    res = run_bass_kernel_spmd(nc, in_maps, core_ids=list(range(NCORES)))
    LAST_RESULTS = res
    out = np.concatenate([r["out"] for r in res.results], axis=0)
    return np.ascontiguousarray(out.reshape(B, C, H, W).astype(np.float32))
